# revision 28
# baseline (speedup 1.0000x reference)
"""KMeansProbSampler Trainium2 kernel (8-core SPMD).

Algorithm (per reference): 8 iterations of
  d2[p,c]   = (h_p - a_c)^2 + (w_p - b_c)^2        (pixel grid 1024x1024, C=128)
  assign[p] = argmin_c max(1, sqrt(d2))            (first-index tie break)
  new[c]    = sum_{p: assign==c} coords_p * heatmap_p / max(1, sqrt(min d2))
No centroid normalization: cluster coords blow up to ~3e5 after iter 0 and
most clusters collapse to exact (0,0) duplicates.

Mapping:
  - Shard pixel rows across 8 cores (128 rows each). A "tile" is one image
    column within the shard: 128 pixels on SBUF partitions.
  - d2 for a tile x all 128 clusters via one K=4 PE matmul:
    lhsT rows [lh', lw', 1, lh'^2+lw'^2] (host precomputed), rhs rows
    [-2la', -2lb', la'^2+lb'^2 (+dup mask), 1] built on device per iteration.
    Coordinates are recentered (h per-core, w per 128-column block).
    Iteration 0 runs fp32 unscaled (coords < 1024, argmin gaps are tight —
    fp16 noise would flip ~1% of boundary pixels); iterations 1+ run fp16
    (1 cyc/row) scaled by LAM_B=1/4096 (cluster coords blow up to ~3e5 and
    argmin gaps are huge, so fp16 noise is irrelevant). The per-iteration
    scale correction (LAM_A/LAM_IT)^2 folds into the rec clamp op.
  - Per 8-tile group (one [128,1024] 2-bank PSUM; matmul start/stop flags
    per BANK — start zeroes the whole target bank): Act copies psum -> s16
    (same dtype as the iteration), DVE segmented min -> m2 (fp32; rounding
    is monotone so min of rounded values == rounded min and is_equal
    matches exactly), then scaled-one-hot soh = (s16 == m2) * rec with
    rec = (1/LAM_A)/max(1,dist), split 3:5 between DVE (fp16 4x mode) and
    Pool (otherwise-idle overflow capacity; GPSIMD cannot touch PSUM).
  - Software pipelining: soh ops are issued at rec-batch completion (32
    tiles), their scatter matmuls one batch later, so the in-order PE
    stream never convoys behind the Pool soh burst.
  - Duplicate clusters get +DUP_MASK on their la'^2+lb'^2 ext entry (device
    epilogue, is_gt so the mask is single-level); first duplicate wins
    exactly like jnp.argmin. Masked entries can round to fp16 inf: min and
    is_equal ignore them, and the multiplying lhsT row is the constant 1.
  - scatter: PE matmul acc[c, 0:2] += soh^T @ vhw (vhw = LAM_A*coords*hm
    fp16, host precomputed), PSUM-accumulated over all 1024 tiles; the
    LAM_A in vhw cancels the 1/LAM_A in rec exactly.
  - per-iteration AllReduce of the [128, 2] partial sums across 8 cores.

TimelineSim (single-core proxy): 2.478 ms baseline -> 1.753 ms.
"""

import os
import sys

import numpy as np

H = 1024
W = 1024
C = 128
N_ITER = 8
NCORES = 8
RPC = H // NCORES  # rows per core
P = 128            # partitions = pixels per tile
NT = W             # tiles (columns) per core
TPB = 128          # tiles per w-block
WG = 8             # tiles per PSUM group ([128, 1024] = two banks)
GPB = 4            # groups per sqrt/recip batch (32 tiles)
LAM_A = 0.125      # coordinate scale, iteration 0
LAM_B = 1.0 / 4096.0  # coordinate scale, iterations 1+
DUP_MASK = 30000.0  # duplicate-cluster mask (scaled-d2 domain)
SOH_DVE = 3        # tiles per group whose one-hot runs on DVE (rest: Pool)

_REPO_CANDIDATES = ("/opt/trn_rl_repo", "/root/.axon_site/_ro/trn_rl_repo")


def _ensure_repo():
    try:
        import concourse  # noqa: F401
        return
    except ImportError:
        pass
    for p in _REPO_CANDIDATES:
        if os.path.isdir(p):
            sys.path.insert(0, p)
            break
    import concourse  # noqa: F401


def build_nc(n_iter: int = N_ITER, nt: int = NT, ncores: int = NCORES,
             timing_mode: bool = False, soh_dve: int = SOH_DVE,
             psd_bufs: int = 2):
    """Build the SPMD Bass program (same program for every core).

    timing_mode=True builds a single-core variant with the AllReduce
    replaced by a local DMA copy so TimelineSim can schedule it.
    """
    _ensure_repo()
    import concourse.bacc as bacc
    import concourse.mybir as mybir
    import concourse.tile as tile

    f32 = mybir.dt.float32
    f16 = mybir.dt.float16
    Alu = mybir.AluOpType
    Act = mybir.ActivationFunctionType
    X = mybir.AxisListType.X

    nblk = (nt + TPB - 1) // TPB
    assert nt % TPB == 0
    gpt = TPB // WG          # groups per block

    nc = bacc.Bacc(
        "TRN2",
        target_bir_lowering=False,
        debug=False,
        num_devices=1 if timing_mode else ncores,
    )

    # ---- I/O ----
    pixTa_d = nc.dram_tensor("pixTa", [4 * nblk, TPB * P], f32,
                             kind="ExternalInput")
    pixTb_d = nc.dram_tensor("pixTb", [4 * nblk, TPB * P], f16,
                             kind="ExternalInput")
    vhw_d = nc.dram_tensor("vhw", [P, 2 * nt], f16, kind="ExternalInput")
    ext0_d = nc.dram_tensor("ext0", [4 * nblk, C], f32, kind="ExternalInput")
    ident_d = nc.dram_tensor("ident", [P, P], f32, kind="ExternalInput")
    ltri_d = nc.dram_tensor("ltri", [P, P], f32, kind="ExternalInput")
    hoff_d = nc.dram_tensor("hoff", [C, 1], f32, kind="ExternalInput")
    out_d = nc.dram_tensor("out", [C, 2], f32, kind="ExternalOutput")

    with tile.TileContext(nc) as tc:
        from contextlib import ExitStack

        with ExitStack() as st:
            const = st.enter_context(tc.tile_pool(name="const", bufs=1))
            stpool = st.enter_context(tc.tile_pool(name="stage", bufs=2))
            stpool32 = st.enter_context(tc.tile_pool(name="stage32", bufs=1))
            spool = st.enter_context(tc.tile_pool(name="s", bufs=9))
            spool32 = st.enter_context(tc.tile_pool(name="s32", bufs=9))
            mpool = st.enter_context(tc.tile_pool(name="m2", bufs=3))
            sohp = st.enter_context(tc.tile_pool(name="soh", bufs=40))
            smal = st.enter_context(tc.tile_pool(name="small", bufs=4))
            eqp = st.enter_context(tc.tile_pool(name="eq", bufs=2))
            psd = st.enter_context(tc.tile_pool(name="psd", bufs=psd_bufs,
                                                space="PSUM"))
            psa = st.enter_context(tc.tile_pool(name="psa", bufs=1, space="PSUM"))
            pse = st.enter_context(tc.tile_pool(name="pse", bufs=1, space="PSUM"))
            dram = st.enter_context(tc.tile_pool(name="dram", bufs=2, space="DRAM"))

            # ---- persistent SBUF state ----
            vhw = const.tile([P, 2 * nt], f16)
            ident = const.tile([P, P], f32)
            ltri = const.tile([P, P], f32)
            hoff = const.tile([C, 1], f32)
            exts = [const.tile([4, C], f16, name=f"ext{b}", tag=f"ext{b}")
                    for b in range(nblk)]
            exts32 = [const.tile([4, C], f32, name=f"e32_{b}", tag=f"e32_{b}")
                      for b in range(nblk)]

            nc.gpsimd.dma_start(vhw[:], vhw_d[:])
            nc.gpsimd.dma_start(ident[:], ident_d[:])
            nc.gpsimd.dma_start(ltri[:], ltri_d[:])
            nc.gpsimd.dma_start(hoff[:], hoff_d[:])
            for b in range(nblk):
                nc.gpsimd.dma_start(exts32[b][:], ext0_d[4 * b:4 * b + 4, :])

            arout_prev = None
            for it in range(n_iter):
                # iteration 0 runs the d2 matmul in fp32 (exact argmin, like
                # the reference); iterations 1+ in fp16 at LAM_B scale
                lam = 1.0 if it == 0 else LAM_B
                pixT_d = pixTa_d if it == 0 else pixTb_d
                it_dt = f32 if it == 0 else f16
                it_exts = exts32 if it == 0 else exts
                it_stpool = stpool32 if it == 0 else stpool
                it_spool = spool32 if it == 0 else spool
                rec_s2 = (LAM_A / lam) ** 2
                acc = psa.tile([C, 2], f32, space="PSUM")

                def emit_soh(batch):
                    """Scaled-one-hot ops for a finished batch (DVE + Pool).

                    Issued immediately at batch completion so Pool/DVE chew
                    through them overlapped with the next batch's d2 phase.
                    """
                    bt0, bs, bm2, brec = batch
                    sohs = []
                    for q, s_q in enumerate(bs):
                        for tau in range(WG):
                            col = q * WG + tau
                            soh = sohp.tile([P, P], f16)
                            eng = (nc.vector if tau < soh_dve
                                   else nc.gpsimd)
                            eng.tensor_scalar(
                                out=soh[:],
                                in0=s_q[:, tau * P:(tau + 1) * P],
                                scalar1=bm2[:, col:col + 1],
                                scalar2=brec[:, col:col + 1],
                                op0=Alu.is_equal,
                                op1=Alu.mult,
                            )
                            sohs.append((bt0 + col, soh))
                    return sohs

                def emit_scatter(sohs):
                    """Scatter matmuls, issued one batch after their sohs so
                    the PE stream never waits on the Pool soh burst."""
                    for t, soh in sohs:
                        nc.tensor.matmul(
                            out=acc[:],
                            lhsT=soh[:],
                            rhs=vhw[:, 2 * t:2 * t + 2],
                            start=(t == 0),
                            stop=(t == nt - 1),
                        )

                pending = None
                for b in range(nblk):
                    stage = it_stpool.tile([4, TPB * P], it_dt, tag="stage")
                    nc.sync.dma_start(stage[:], pixT_d[4 * b:4 * b + 4, :])
                    for gl in range(gpt):
                        t0 = b * TPB + gl * WG
                        psum_d = psd.tile([P, WG * P], f32, space="PSUM")
                        for tau in range(WG):
                            loc = gl * WG + tau
                            # start on the first write to EACH psum bank
                            # (a [128,1024] f32 tile spans two banks)
                            nc.tensor.matmul(
                                out=psum_d[:, tau * P:(tau + 1) * P],
                                lhsT=stage[:, loc * P:(loc + 1) * P],
                                rhs=it_exts[b][:],
                                start=(tau % 4 == 0),
                                stop=(tau % 4 == 3),
                            )
                        s16 = it_spool.tile([P, WG * P], it_dt)
                        nc.scalar.copy(out=s16[:], in_=psum_d[:])

                        gg = gl % GPB
                        if gg == 0:
                            m2 = mpool.tile([P, GPB * WG], f32, tag="m2")
                            rec = mpool.tile([P, GPB * WG], f32, tag="rec")
                            sq = mpool.tile([P, GPB * WG], f32, tag="sq")
                            batch_s = []
                            batch_t0 = t0
                        batch_s.append(s16)
                        nc.vector.tensor_reduce(
                            out=m2[:, gg * WG:(gg + 1) * WG],
                            in_=s16[:].rearrange("p (n x) -> p n x", x=P),
                            axis=X,
                            op=Alu.min,
                        )
                        if gg == GPB - 1:
                            # batched rec = (1/LAM_A)/max(1, dist) for 32 tiles
                            nc.vector.tensor_scalar(
                                out=sq[:], in0=m2[:], scalar1=lam * lam,
                                scalar2=rec_s2, op0=Alu.max, op1=Alu.mult,
                            )
                            nc.scalar.activation(out=sq[:], in_=sq[:],
                                                 func=Act.Sqrt)
                            nc.vector.reciprocal(out=rec[:], in_=sq[:])
                            sohs = emit_soh((batch_t0, batch_s, m2, rec))
                            if pending is not None:
                                emit_scatter(pending)
                            pending = sohs
                emit_scatter(pending)

                # ---- partial [C,2] -> AllReduce ----
                part = smal.tile([C, 2], f32, tag="part")
                nc.scalar.copy(out=part[:], in_=acc[:])
                arin = dram.tile([C, 2], f32)
                arout = dram.tile([C, 2], f32)
                nc.sync.dma_start(arin[:], part[:])
                if timing_mode:
                    nc.sync.dma_start(arout[:], arin[:])
                else:
                    nc.gpsimd.collective_compute(
                        "AllReduce",
                        Alu.add,
                        replica_groups=[list(range(ncores))],
                        ins=[arin[:].opt()],
                        outs=[arout[:].opt()],
                    )
                arout_prev = arout

                if it == n_iter - 1:
                    break

                # ---- epilogue: rebuild per-block ext (LAM_B domain) ----
                ncs = smal.tile([C, 2], f32, tag="ncs")
                nc.sync.dma_start(ncs[:], arout[:])

                # la'/basec chain first: independent of the dup detection
                apc = smal.tile([C, 1], f32, tag="apc")
                nc.vector.tensor_scalar(out=apc[:], in0=ncs[:, 0:1],
                                        scalar1=hoff[:, 0:1], scalar2=LAM_B,
                                        op0=Alu.subtract, op1=Alu.mult)
                basec = smal.tile([C, 1], f32, tag="basec")
                nc.vector.tensor_tensor(out=basec[:], in0=apc[:], in1=apc[:],
                                        op=Alu.mult)

                # broadcast a and b along free dim: bc[i, j] = coord_j
                abc = pse.tile([C, C], f32, space="PSUM", tag="bca")
                nc.tensor.transpose(
                    out=abc[:], in_=ncs[:, 0:1].to_broadcast([C, C]),
                    identity=ident[:],
                )
                bbc = pse.tile([C, C], f32, space="PSUM", tag="bcb")
                nc.tensor.transpose(
                    out=bbc[:], in_=ncs[:, 1:2].to_broadcast([C, C]),
                    identity=ident[:],
                )
                eqa = eqp.tile([C, C], f32, tag="eqa")
                nc.vector.tensor_scalar(
                    out=eqa[:], in0=abc[:], scalar1=ncs[:, 0:1], scalar2=None,
                    op0=Alu.is_equal,
                )
                eqb = eqp.tile([C, C], f32, tag="eqb")
                nc.vector.tensor_scalar(
                    out=eqb[:], in0=bbc[:], scalar1=ncs[:, 1:2], scalar2=None,
                    op0=Alu.is_equal,
                )
                nc.vector.tensor_tensor(out=eqa[:], in0=eqa[:], in1=eqb[:],
                                        op=Alu.mult)
                nc.vector.tensor_tensor(out=eqa[:], in0=eqa[:], in1=ltri[:],
                                        op=Alu.mult)
                # cfs[i] = DUP_MASK iff cluster i has an earlier duplicate
                cfs = smal.tile([C, 1], f32, tag="cfs")
                nc.vector.tensor_reduce(out=cfs[:], in_=eqa[:], axis=X,
                                        op=Alu.add)
                nc.vector.tensor_scalar(
                    out=cfs[:], in0=cfs[:], scalar1=0.5, scalar2=DUP_MASK,
                    op0=Alu.is_gt, op1=Alu.mult,
                )
                nc.vector.tensor_tensor(out=basec[:], in0=basec[:], in1=cfs[:],
                                        op=Alu.add)
                for b in range(nblk):
                    w0 = float(b * TPB + 64)
                    eng = nc.vector if b % 2 == 0 else nc.gpsimd
                    extt = eqp.tile([C, 4], f32, tag=f"extt{b % 2}")
                    bpc = smal.tile([C, 1], f32, tag=f"bpc{b % 2}")
                    b2c = smal.tile([C, 1], f32, tag=f"b2c{b % 2}")
                    eng.tensor_scalar(
                        out=extt[:, 0:1], in0=apc[:], scalar1=-2.0,
                        scalar2=None, op0=Alu.mult,
                    )
                    eng.tensor_scalar(
                        out=extt[:, 1:2], in0=ncs[:, 1:2], scalar1=w0,
                        scalar2=-2.0 * LAM_B, op0=Alu.subtract, op1=Alu.mult,
                    )
                    eng.tensor_scalar(
                        out=bpc[:], in0=ncs[:, 1:2], scalar1=w0, scalar2=LAM_B,
                        op0=Alu.subtract, op1=Alu.mult,
                    )
                    eng.tensor_tensor(out=b2c[:], in0=bpc[:], in1=bpc[:],
                                      op=Alu.mult)
                    eng.tensor_tensor(out=extt[:, 2:3], in0=b2c[:],
                                      in1=basec[:], op=Alu.add)
                    eng.memset(extt[:, 3:4], 1.0)
                    extp = pse.tile([4, C], f32, space="PSUM", tag="extp")
                    nc.tensor.transpose(out=extp[:], in_=extt[:],
                                        identity=ident[:])
                    nc.scalar.copy(out=exts[b][:], in_=extp[:])

            # final output
            nc.sync.dma_start(out_d[:], arout_prev[:])

    nc.compile()
    return nc


def make_core_inputs(core: int, clusters: np.ndarray, heatmap: np.ndarray,
                     nt: int = NT):
    """Host-precomputed per-core constant tables (fp16, LAM-scaled)."""
    nblk = nt // TPB
    r0 = core * RPC
    hoff = np.float32(r0 + 64)
    hs = (np.arange(P, dtype=np.float32) + np.float32(r0))

    def make_pixT(lam, dtype):
        lam = np.float32(lam)
        hp = ((hs - hoff) * lam).astype(np.float32)
        pixT = np.zeros((4 * nblk, TPB * P), np.float32)
        for b in range(nblk):
            w0 = np.float32(b * TPB + 64)
            for tau in range(TPB):
                t = b * TPB + tau
                wp = np.float32((np.float32(t) - w0) * lam)
                sl = slice(tau * P, (tau + 1) * P)
                pixT[4 * b + 0, sl] = hp
                pixT[4 * b + 1, sl] = wp
                pixT[4 * b + 2, sl] = 1.0
                pixT[4 * b + 3, sl] = (hp * hp + wp * wp).astype(np.float32)
        return pixT.astype(dtype)

    hm = heatmap[r0:r0 + RPC, :nt].astype(np.float32)
    vhw = np.empty((P, 2 * nt), np.float32)
    vhw[:, 0::2] = (hs[:, None] * hm * np.float32(LAM_A)).astype(np.float32)
    vhw[:, 1::2] = (np.arange(nt, dtype=np.float32)[None, :] * hm
                    * np.float32(LAM_A)).astype(np.float32)

    a = clusters[:, 0].astype(np.float32)
    b_ = clusters[:, 1].astype(np.float32)
    ext0 = np.zeros((4 * nblk, C), np.float32)
    ap = (a - hoff).astype(np.float32)
    for b in range(nblk):
        w0 = np.float32(b * TPB + 64)
        bp = (b_ - w0).astype(np.float32)
        ext0[4 * b + 0] = (np.float32(-2.0) * ap).astype(np.float32)
        ext0[4 * b + 1] = (np.float32(-2.0) * bp).astype(np.float32)
        ext0[4 * b + 2] = (ap * ap + bp * bp).astype(np.float32)
        ext0[4 * b + 3] = 1.0

    return {
        "pixTa": make_pixT(1.0, np.float32),
        "pixTb": make_pixT(LAM_B, np.float16),
        "vhw": vhw.astype(np.float16),
        "ext0": ext0.astype(np.float32),
        "ident": np.eye(P, dtype=np.float32),
        "ltri": np.tril(np.ones((P, P), np.float32), -1),
        "hoff": np.full((C, 1), hoff, dtype=np.float32),
    }


_NC_CACHE = {}


def kernel(clusters: np.ndarray, heatmap: np.ndarray) -> np.ndarray:
    _ensure_repo()
    from concourse.bass_utils import run_bass_kernel_spmd

    clusters = np.asarray(clusters, np.float32)
    heatmap = np.asarray(heatmap, np.float32)

    key = (N_ITER, NT)
    if key not in _NC_CACHE:
        _NC_CACHE[key] = build_nc()
    nc = _NC_CACHE[key]

    in_maps = [make_core_inputs(k, clusters, heatmap) for k in range(NCORES)]
    res = run_bass_kernel_spmd(nc, in_maps, list(range(NCORES)))
    return np.asarray(res.results[0]["out"], np.float32)


if __name__ == "__main__":
    _ensure_repo()
    nc = build_nc(n_iter=int(sys.argv[1]) if len(sys.argv) > 1 else 1,
                  nt=int(sys.argv[2]) if len(sys.argv) > 2 else 128)
    print("built + compiled OK")
